# revision 14
# baseline (speedup 1.0000x reference)
"""MiniMax sparse-MoE Trainium2 kernel (expert-parallel over 8 NeuronCores).

Reference computes a dense MoE: router top-2 softmax over E=8 experts, then
out = sum_e combine[t,e] * SwiGLU_e(x[t]).  combine is zero except at the
top-2 experts, so only the top-2 expert MLPs per token are computed.

Sharding: expert-parallel.  Core e owns expert e's weights.  The host does
the routing (tiny: [2048,8] logits) and the all-to-all dispatch/combine
(gather tokens per expert, scatter-add the per-expert outputs), which is the
natural host-side realization of the sharding_hint's a2a.  The device does
all the heavy FLOPs: per core, three [C,2048]x[2048,5632]-class GEMMs with a
fused SwiGLU, with every operand pre-laid-out on the host so each DMA is a
contiguous per-partition stream and no on-device transposes are needed.

Device kernel layout (per core, capacity C tokens, P=128):
  x_t   [128, DP, C]   x gathered+transposed: x_t[p,dp,c] = x[tok_c, dp*128+p]
  wg_t  [IP, 128, DP, 128]  wg_t[ic,p,dp,j] = w_gate[e][ic*128+j, dp*128+p]
  wu_t  same layout as wg_t
  wd_t  [DP, 128, IP, 128]  wd_t[dc,p,ip,j] = w_down[e][dc*128+j, ip*128+p]
  scl   [128, C]       combine weight per token (replicated over partitions)
  y_t   [128, DP, C]   output, y_t[p,dc,c] = out_e[tok_c, dc*128+p]

Compute (everything feature-major so no transposes anywhere):
  gT[i,c] = sum_d wg[i,d] x[c,d]   (lhsT = wg tile [d,i], rhs = x_t [d,c])
  h = silu(gT) * uT                (ACT + DVE, h resident in SBUF)
  yT[d,c] = sum_i wd[d,i] h[i,c]   (lhsT = wd tile [i,d], rhs = h [i,c])
  y *= scl                          (combine weight; padding rows have 0)
I is processed in NQ resident quarters to fit h in SBUF; y accumulates in
SBUF across quarters.
"""

import os
import sys

sys.path.insert(0, "/opt/trn_rl_repo")

import numpy as np

import concourse.bass as bass
import concourse.mybir as mybir
import concourse.tile as tile
from concourse.bass_utils import run_bass_kernel_spmd
from concourse.vector_clock import ScopedClock

T, D, I, E, K = 2048, 2048, 5632, 8, 2
P = 128
DP = D // P            # 16
IP = I // P            # 44
NQ = 4                 # I-quarters resident in SBUF
IPQ = IP // NQ         # 11
N_CORES = 8

# "fp32" (exact, 4 cyc/row), "fp32r" (reduced-precision matmul, 1 cyc/row),
# "bf16" (inputs cast to bf16, half DMA traffic)
DTYPE_MODE = os.environ.get("MOE_DTYPE_MODE", "fp32r")


class _SplitDrainTileContext(tile.TileContext):
    """This container's walrus rejects >~2 sync waits on the kernel-tail
    Drain ("Too many sync wait commands").  Split the drain's waits onto
    single-wait NOPs emitted just before it on the same engine."""

    def _drain_and_barrier(self, tick_clock, wait_clock):
        nc = self.nc
        probe = nc.sync.nop()
        wait_clock.add_sem_waits(
            probe.ins, ScopedClock({None: tick_clock.global_clock})
        )
        waits = list(probe.ins.sync_info.on_wait or [])
        probe.ins.sync_info.on_wait = waits[:1]
        for w in waits[1:]:
            nop = nc.sync.nop()
            if nop.ins.sync_info is None:
                nop.ins.sync_info = mybir.SyncInfo(on_wait=[w], on_update=[])
            else:
                nop.ins.sync_info.on_wait = [w]
        nc.sync.drain()
        nc.all_engine_barrier()
        assert self.sems is not None
        popped = nc._tile_sem_poison_stack.pop()
        assert popped is self._sem_poison
        nc.clear_and_free_semaphores(list(self.sems.allocated().values()))
        nc.all_engine_barrier()


def _split_excess_waits(nc, cap=1):
    """This container's walrus codegen accepts only ~1 sync-wait command per
    instruction (a Matmult with 2 waits dies in setupSyncWait).  Hoist excess
    waits onto same-engine NOPs placed immediately before the instruction —
    the engine executes in order, so the guarantee is identical."""
    for blk in nc.m.functions[0].blocks:
        new = []
        for inst in blk.instructions:
            si = getattr(inst, "sync_info", None)
            waits = list(si.on_wait) if si is not None and si.on_wait else []
            if len(waits) > cap:
                for k, w in enumerate(waits[cap:]):
                    new.append(
                        mybir.InstNoOp(
                            name=f"{inst.name}-wsplit{k}",
                            engine=inst.engine,
                            bass_nofuse=True,
                            sync_info=mybir.SyncInfo(on_wait=[w], on_update=[]),
                        )
                    )
                si.on_wait = waits[:cap]
            new.append(inst)
        if len(new) != len(blk.instructions):
            blk.instructions = new


def _col_blocks(C):
    """Split the token axis into PSUM-bank-sized column blocks (<=512).

    Blocks are BALANCED rather than 512+tail: fp32r matmuls drop to 4
    cyc/row when the moving dim is <256, so e.g. 576 must become 288+288,
    not 512+64."""
    nb = -(-C // 512)
    base = -(-(C // nb) // 32) * 32
    blocks = []
    off = 0
    while off < C:
        bw = min(base, C - off)
        blocks.append((off, bw))
        off += bw
    return blocks


def build_kernel(C, dtype_mode=DTYPE_MODE, reps=1):
    f32 = mybir.dt.float32
    # float32r: same 4-byte storage as fp32 (numpy side is float32), but the
    # BIR verifier requires every producer feeding an fp32r matmul to emit
    # fp32r, so declare the DRAM params and SBUF tiles holding matmul
    # operands (x, weights, h) as float32r end to end.
    if dtype_mode == "bf16":
        in_dt = mybir.dt.bfloat16
    elif dtype_mode == "fp32r":
        in_dt = mybir.dt.float32r
    else:
        in_dt = f32

    def mm(ap):
        return ap

    nc = bass.Bass()
    x_t = nc.declare_dram_parameter("x_t", [P, DP, C], in_dt, isOutput=False)
    wg_t = nc.declare_dram_parameter("wg_t", [IP, P, DP, P], in_dt, isOutput=False)
    wu_t = nc.declare_dram_parameter("wu_t", [IP, P, DP, P], in_dt, isOutput=False)
    wd_t = nc.declare_dram_parameter("wd_t", [DP, P, IP, P], in_dt, isOutput=False)
    scl = nc.declare_dram_parameter("scl", [P, C], f32, isOutput=False)
    y_t = nc.declare_dram_parameter("y_t", [P, DP, C], f32, isOutput=True)

    blocks = _col_blocks(C)
    Silu = mybir.ActivationFunctionType.Silu

    with _SplitDrainTileContext(nc) as tc:
        with (
            tc.tile_pool(name="xpool", bufs=1) as xpool,
            tc.tile_pool(name="hpool", bufs=2) as hpool,
            tc.tile_pool(name="ypool", bufs=1) as ypool,
            tc.tile_pool(name="w1pool", bufs=3) as w1pool,
            tc.tile_pool(name="wdpool", bufs=2) as wdpool,
            tc.tile_pool(name="tmppool", bufs=3) as tmppool,
            tc.tile_pool(name="pgu", bufs=3, space="PSUM") as pgu,
            tc.tile_pool(name="pyp", bufs=2, space="PSUM") as pyp,
        ):
            x_sb = xpool.tile([P, DP, C], in_dt)
            nc.sync.dma_start(x_sb[:], x_t[:])
            scl_sb = xpool.tile([P, C], f32, tag="scl")
            nc.sync.dma_start(scl_sb[:], scl[:])
            y_sb = ypool.tile([P, DP, C], f32)

            for _rep, q in ((r, qq) for r in range(reps) for qq in range(NQ)):
                # double-buffered per quarter: layer1(q+1) can fill while
                # layer3(q) drains
                h_sb = hpool.tile([P, IPQ, C], in_dt, name="h_sb")
                # ---- layer 1+2: gT/uT for this I-quarter, fused SwiGLU → h
                for il in range(IPQ):
                    ic = q * IPQ + il
                    wg_sb = w1pool.tile([P, DP, P], in_dt, tag="wg")
                    nc.sync.dma_start(wg_sb[:], wg_t[ic])
                    wu_sb = w1pool.tile([P, DP, P], in_dt, tag="wu")
                    nc.sync.dma_start(wu_sb[:], wu_t[ic])
                    for off, bw in blocks:
                        blk = slice(off, off + bw)
                        pg = pgu.tile([P, 512], f32, tag="pg", name="pg")[:, :bw]
                        pu = pgu.tile([P, 512], f32, tag="pu", name="pu")[:, :bw]
                        for dp in range(DP):
                            nc.tensor.matmul(
                                pg,
                                mm(wg_sb[:, dp]),
                                mm(x_sb[:, dp, blk]),
                                start=(dp == 0),
                                stop=(dp == DP - 1),
                            )
                        for dp in range(DP):
                            nc.tensor.matmul(
                                pu,
                                mm(wu_sb[:, dp]),
                                mm(x_sb[:, dp, blk]),
                                start=(dp == 0),
                                stop=(dp == DP - 1),
                            )
                        tmp = tmppool.tile([P, 512], f32, tag="silu", name="silu")[:, :bw]
                        nc.scalar.activation(tmp, pg, Silu)
                        nc.vector.tensor_mul(
                            out=h_sb[:, il, blk], in0=tmp, in1=pu
                        )

                # ---- layer 3: partial down-proj for this quarter → y_sb
                for dc in range(DP):
                    wd_sb = wdpool.tile([P, IPQ, P], in_dt, tag="wd")
                    nc.sync.dma_start(
                        wd_sb[:], wd_t[dc, :, q * IPQ : (q + 1) * IPQ, :]
                    )
                    for off, bw in blocks:
                        blk = slice(off, off + bw)
                        py = pyp.tile([P, 512], f32, tag="py", name="py")[:, :bw]
                        for il in range(IPQ):
                            nc.tensor.matmul(
                                py,
                                mm(wd_sb[:, il]),
                                mm(h_sb[:, il, blk]),
                                start=(il == 0),
                                stop=(il == IPQ - 1),
                            )
                        if q == 0:
                            nc.scalar.copy(y_sb[:, dc, blk], py)
                        else:
                            nc.vector.tensor_add(
                                out=y_sb[:, dc, blk], in0=y_sb[:, dc, blk], in1=py
                            )
                        if q == NQ - 1:
                            nc.vector.tensor_mul(
                                out=y_sb[:, dc, blk],
                                in0=y_sb[:, dc, blk],
                                in1=scl_sb[:, blk],
                            )
                            nc.sync.dma_start(y_t[:, dc, blk], y_sb[:, dc, blk])
    _split_excess_waits(nc)
    return nc


def _capacity(maxc):
    """Token capacity per expert: exact max count, rounded up a little for
    DMA alignment (no need for a 128 multiple — the tail column block just
    gets a narrower matmul)."""
    return max(-(-maxc // 32) * 32, 128)


def _route(x, gate_w):
    """Host router: float64 logits, top-2, softmax.  Returns per-expert
    (token_idx, weight) lists."""
    logits = x.astype(np.float64) @ gate_w.astype(np.float64).T
    order = np.argsort(-logits, axis=1, kind="stable")[:, :K]      # [T, K]
    top = np.take_along_axis(logits, order, axis=1)                # [T, K]
    m = top.max(axis=1, keepdims=True)
    ex = np.exp(top - m)
    rw = (ex / ex.sum(axis=1, keepdims=True)).astype(np.float32)   # [T, K]
    idx_e, w_e = [], []
    for e in range(E):
        tok, slot = np.nonzero(order == e)
        idx_e.append(tok.astype(np.int64))
        w_e.append(rw[tok, slot])
    return idx_e, w_e


def prepare_in_maps(x, w_gate, w_up, w_down, idx_e, w_e, C):
    """Host-side dispatch: gather each expert's tokens and pre-arrange every
    tensor into the exact SBUF tile layout the device kernel streams."""
    if DTYPE_MODE == "bf16":
        import ml_dtypes
        in_np = ml_dtypes.bfloat16
    else:
        in_np = np.float32
    in_maps = []
    for e in range(E):
        n = len(idx_e[e])
        idx = np.zeros(C, dtype=np.int64)
        idx[:n] = idx_e[e]
        s = np.zeros(C, dtype=np.float32)
        s[:n] = w_e[e]

        xe = x[idx]                                       # [C, D]
        x_t = np.ascontiguousarray(
            xe.reshape(C, DP, P).transpose(2, 1, 0), dtype=in_np
        )
        wg_t = np.ascontiguousarray(
            w_gate[e].reshape(IP, P, DP, P).transpose(0, 3, 2, 1), dtype=in_np
        )
        wu_t = np.ascontiguousarray(
            w_up[e].reshape(IP, P, DP, P).transpose(0, 3, 2, 1), dtype=in_np
        )
        wd_t = np.ascontiguousarray(
            w_down[e].reshape(DP, P, IP, P).transpose(0, 3, 2, 1), dtype=in_np
        )
        scl = np.ascontiguousarray(np.broadcast_to(s, (P, C)))
        in_maps.append(
            {"x_t": x_t, "wg_t": wg_t, "wu_t": wu_t, "wd_t": wd_t, "scl": scl}
        )
    return in_maps


def kernel(x, gate_w, w_gate, w_up, w_down):
    x = np.ascontiguousarray(np.asarray(x, dtype=np.float32))
    gate_w = np.asarray(gate_w, dtype=np.float32)
    w_gate = np.asarray(w_gate, dtype=np.float32)
    w_up = np.asarray(w_up, dtype=np.float32)
    w_down = np.asarray(w_down, dtype=np.float32)

    idx_e, w_e = _route(x, gate_w)
    C = _capacity(max(len(i) for i in idx_e))

    in_maps = prepare_in_maps(x, w_gate, w_up, w_down, idx_e, w_e, C)
    # Retry on transient device wedges (NRT_EXEC_UNIT_UNRECOVERABLE has been
    # observed sporadically on this fabric; a fresh dispatch recovers).
    last = None
    for _attempt in range(3):
        try:
            nc = build_kernel(C)
            res = run_bass_kernel_spmd(
                nc, in_maps, core_ids=list(range(N_CORES))
            )
            break
        except Exception as exc:  # noqa: BLE001
            last = exc
    else:
        raise last

    out = np.zeros((T, D), dtype=np.float32)
    for e in range(E):
        n = len(idx_e[e])
        if n == 0:
            continue
        y_t = res.results[e]["y_t"]                       # [P, DP, C]
        ye = y_t.transpose(2, 1, 0).reshape(C, D)[:n]     # [n, D]
        out[idx_e[e]] += ye
    return out


if __name__ == "__main__":
    rng = np.random.default_rng(0)
    # tiny smoke of the host routing path only
    print(_route(rng.standard_normal((16, D), dtype=np.float32),
                 rng.standard_normal((E, D), dtype=np.float32) * 0.02)[0])


# revision 15
# speedup vs baseline: 1.4031x; 1.4031x over previous
"""MiniMax sparse-MoE Trainium2 kernel (expert-parallel over 8 NeuronCores).

Reference computes a dense MoE: router top-2 softmax over E=8 experts, then
out = sum_e combine[t,e] * SwiGLU_e(x[t]).  combine is zero except at the
top-2 experts, so only the top-2 expert MLPs per token are computed.

Sharding: expert-parallel.  Core e owns expert e's weights.  The host does
the routing (tiny: [2048,8] logits) and the all-to-all dispatch/combine
(gather tokens per expert, scatter-add the per-expert outputs), which is the
natural host-side realization of the sharding_hint's a2a.  The device does
all the heavy FLOPs: per core, three [C,2048]x[2048,5632]-class GEMMs with a
fused SwiGLU, with every operand pre-laid-out on the host so each DMA is a
contiguous per-partition stream and no on-device transposes are needed.

Device kernel layout (per core, capacity C tokens, P=128):
  x_t   [128, DP, C]   x gathered+transposed: x_t[p,dp,c] = x[tok_c, dp*128+p]
  wg_t  [IP, 128, DP, 128]  wg_t[ic,p,dp,j] = w_gate[e][ic*128+j, dp*128+p]
  wu_t  same layout as wg_t
  wd_t  [DP, 128, IP, 128]  wd_t[dc,p,ip,j] = w_down[e][dc*128+j, ip*128+p]
  scl   [128, C]       combine weight per token (replicated over partitions)
  y_t   [128, DP, C]   output, y_t[p,dc,c] = out_e[tok_c, dc*128+p]

Compute (everything feature-major so no transposes anywhere):
  gT[i,c] = sum_d wg[i,d] x[c,d]   (lhsT = wg tile [d,i], rhs = x_t [d,c])
  h = silu(gT) * uT                (ACT + DVE, h resident in SBUF)
  yT[d,c] = sum_i wd[d,i] h[i,c]   (lhsT = wd tile [i,d], rhs = h [i,c])
  y *= scl                          (combine weight; padding rows have 0)
I is processed in NQ resident quarters to fit h in SBUF; y accumulates in
SBUF across quarters.
"""

import os
import sys

sys.path.insert(0, "/opt/trn_rl_repo")

import numpy as np

import concourse.bass as bass
import concourse.mybir as mybir
import concourse.tile as tile
from concourse.bass_utils import run_bass_kernel_spmd
from concourse.vector_clock import ScopedClock

T, D, I, E, K = 2048, 2048, 5632, 8, 2
P = 128
DP = D // P            # 16
IP = I // P            # 44
NQ = 4                 # I-quarters resident in SBUF
IPQ = IP // NQ         # 11
N_CORES = 8

# "fp32" (exact, 4 cyc/row), "fp32r" (reduced-precision matmul, 1 cyc/row),
# "bf16" (inputs cast to bf16, half DMA traffic)
DTYPE_MODE = os.environ.get("MOE_DTYPE_MODE", "fp32r")


class _SplitDrainTileContext(tile.TileContext):
    """This container's walrus rejects >~2 sync waits on the kernel-tail
    Drain ("Too many sync wait commands").  Split the drain's waits onto
    single-wait NOPs emitted just before it on the same engine."""

    def _drain_and_barrier(self, tick_clock, wait_clock):
        nc = self.nc
        probe = nc.sync.nop()
        wait_clock.add_sem_waits(
            probe.ins, ScopedClock({None: tick_clock.global_clock})
        )
        waits = list(probe.ins.sync_info.on_wait or [])
        probe.ins.sync_info.on_wait = waits[:1]
        for w in waits[1:]:
            nop = nc.sync.nop()
            if nop.ins.sync_info is None:
                nop.ins.sync_info = mybir.SyncInfo(on_wait=[w], on_update=[])
            else:
                nop.ins.sync_info.on_wait = [w]
        nc.sync.drain()
        nc.all_engine_barrier()
        assert self.sems is not None
        popped = nc._tile_sem_poison_stack.pop()
        assert popped is self._sem_poison
        nc.clear_and_free_semaphores(list(self.sems.allocated().values()))
        nc.all_engine_barrier()


def _split_excess_waits(nc, cap=1):
    """This container's walrus codegen accepts only ~1 sync-wait command per
    instruction (a Matmult with 2 waits dies in setupSyncWait).  Hoist excess
    waits onto same-engine NOPs placed immediately before the instruction —
    the engine executes in order, so the guarantee is identical."""
    for blk in nc.m.functions[0].blocks:
        new = []
        for inst in blk.instructions:
            si = getattr(inst, "sync_info", None)
            waits = list(si.on_wait) if si is not None and si.on_wait else []
            if len(waits) > cap:
                for k, w in enumerate(waits[cap:]):
                    new.append(
                        mybir.InstNoOp(
                            name=f"{inst.name}-wsplit{k}",
                            engine=inst.engine,
                            bass_nofuse=True,
                            sync_info=mybir.SyncInfo(on_wait=[w], on_update=[]),
                        )
                    )
                si.on_wait = waits[:cap]
            new.append(inst)
        if len(new) != len(blk.instructions):
            blk.instructions = new


def _col_blocks(C):
    """Split the token axis into PSUM-bank-sized column blocks (<=512).

    Blocks are BALANCED rather than 512+tail: fp32r matmuls drop to 4
    cyc/row when the moving dim is <256, so e.g. 576 must become 288+288,
    not 512+64."""
    nb = -(-C // 512)
    base = -(-(C // nb) // 32) * 32
    blocks = []
    off = 0
    while off < C:
        bw = min(base, C - off)
        blocks.append((off, bw))
        off += bw
    return blocks


def build_kernel(C, dtype_mode=DTYPE_MODE, reps=1):
    f32 = mybir.dt.float32
    # float32r: same 4-byte storage as fp32 (numpy side is float32), but the
    # BIR verifier requires every producer feeding an fp32r matmul to emit
    # fp32r, so declare the DRAM params and SBUF tiles holding matmul
    # operands (x, weights, h) as float32r end to end.
    if dtype_mode == "bf16":
        in_dt = mybir.dt.bfloat16
    elif dtype_mode == "fp32r":
        in_dt = mybir.dt.float32r
    else:
        in_dt = f32

    def mm(ap):
        return ap

    nc = bass.Bass()
    x_t = nc.declare_dram_parameter("x_t", [P, DP, C], in_dt, isOutput=False)
    wg_t = nc.declare_dram_parameter("wg_t", [IP, P, DP, P], in_dt, isOutput=False)
    wu_t = nc.declare_dram_parameter("wu_t", [IP, P, DP, P], in_dt, isOutput=False)
    wd_t = nc.declare_dram_parameter("wd_t", [DP, P, IP, P], in_dt, isOutput=False)
    scl = nc.declare_dram_parameter("scl", [P, C], f32, isOutput=False)
    y_t = nc.declare_dram_parameter("y_t", [P, DP, C], f32, isOutput=True)

    blocks = _col_blocks(C)
    Silu = mybir.ActivationFunctionType.Silu

    with _SplitDrainTileContext(nc) as tc:
        with (
            tc.tile_pool(name="xpool", bufs=1) as xpool,
            tc.tile_pool(name="hpool", bufs=2) as hpool,
            tc.tile_pool(name="ypool", bufs=1) as ypool,
            tc.tile_pool(name="w1pool", bufs=3) as w1pool,
            tc.tile_pool(name="wdpool", bufs=2) as wdpool,
            tc.tile_pool(name="tmppool", bufs=3) as tmppool,
            tc.tile_pool(name="pgu", bufs=3, space="PSUM") as pgu,
            tc.tile_pool(name="pyp", bufs=2, space="PSUM") as pyp,
        ):
            x_sb = xpool.tile([P, DP, C], in_dt)
            nc.sync.dma_start(x_sb[:], x_t[:])
            scl_sb = xpool.tile([P, C], f32, tag="scl")
            nc.sync.dma_start(scl_sb[:], scl[:])
            y_sb = ypool.tile([P, DP, C], f32)

            for _rep, q in ((r, qq) for r in range(reps) for qq in range(NQ)):
                # double-buffered per quarter: layer1(q+1) can fill while
                # layer3(q) drains
                h_sb = hpool.tile([P, IPQ, C], in_dt, name="h_sb")
                # ---- layer 1+2: gT/uT for this I-quarter, fused SwiGLU → h
                for il in range(IPQ):
                    ic = q * IPQ + il
                    wg_sb = w1pool.tile([P, DP, P], in_dt, tag="wg")
                    nc.sync.dma_start(wg_sb[:], wg_t[ic])
                    wu_sb = w1pool.tile([P, DP, P], in_dt, tag="wu")
                    nc.sync.dma_start(wu_sb[:], wu_t[ic])
                    for off, bw in blocks:
                        blk = slice(off, off + bw)
                        pg = pgu.tile([P, 512], f32, tag="pg", name="pg")[:, :bw]
                        pu = pgu.tile([P, 512], f32, tag="pu", name="pu")[:, :bw]
                        for dp in range(DP):
                            nc.tensor.matmul(
                                pg,
                                mm(wg_sb[:, dp]),
                                mm(x_sb[:, dp, blk]),
                                start=(dp == 0),
                                stop=(dp == DP - 1),
                            )
                        for dp in range(DP):
                            nc.tensor.matmul(
                                pu,
                                mm(wu_sb[:, dp]),
                                mm(x_sb[:, dp, blk]),
                                start=(dp == 0),
                                stop=(dp == DP - 1),
                            )
                        tmp = tmppool.tile([P, 512], f32, tag="silu", name="silu")[:, :bw]
                        nc.scalar.activation(tmp, pg, Silu)
                        nc.vector.tensor_mul(
                            out=h_sb[:, il, blk], in0=tmp, in1=pu
                        )

                # ---- layer 3: partial down-proj for this quarter → y_sb
                for dc in range(DP):
                    wd_sb = wdpool.tile([P, IPQ, P], in_dt, tag="wd")
                    nc.sync.dma_start(
                        wd_sb[:], wd_t[dc, :, q * IPQ : (q + 1) * IPQ, :]
                    )
                    for off, bw in blocks:
                        blk = slice(off, off + bw)
                        py = pyp.tile([P, 512], f32, tag="py", name="py")[:, :bw]
                        for il in range(IPQ):
                            nc.tensor.matmul(
                                py,
                                mm(wd_sb[:, il]),
                                mm(h_sb[:, il, blk]),
                                start=(il == 0),
                                stop=(il == IPQ - 1),
                            )
                        if q == 0:
                            nc.scalar.copy(y_sb[:, dc, blk], py)
                        else:
                            nc.vector.tensor_add(
                                out=y_sb[:, dc, blk], in0=y_sb[:, dc, blk], in1=py
                            )
                        if q == NQ - 1:
                            nc.vector.tensor_mul(
                                out=y_sb[:, dc, blk],
                                in0=y_sb[:, dc, blk],
                                in1=scl_sb[:, blk],
                            )
                            nc.sync.dma_start(y_t[:, dc, blk], y_sb[:, dc, blk])
    _split_excess_waits(nc)
    return nc


def _capacity(maxc):
    """Token capacity per expert: exact max count, rounded up to an even
    number of columns (no need for a 128 multiple — the column blocks are
    balanced and every padded column costs PE streaming cycles, so pad as
    little as possible)."""
    return max(-(-maxc // 2) * 2, 128)


def _route(x, gate_w):
    """Host router: float64 logits, top-2, softmax.  Returns per-expert
    (token_idx, weight) lists."""
    logits = x.astype(np.float64) @ gate_w.astype(np.float64).T
    order = np.argsort(-logits, axis=1, kind="stable")[:, :K]      # [T, K]
    top = np.take_along_axis(logits, order, axis=1)                # [T, K]
    m = top.max(axis=1, keepdims=True)
    ex = np.exp(top - m)
    rw = (ex / ex.sum(axis=1, keepdims=True)).astype(np.float32)   # [T, K]
    idx_e, w_e = [], []
    for e in range(E):
        tok, slot = np.nonzero(order == e)
        idx_e.append(tok.astype(np.int64))
        w_e.append(rw[tok, slot])
    return idx_e, w_e


def prepare_in_maps(x, w_gate, w_up, w_down, idx_e, w_e, C):
    """Host-side dispatch: gather each expert's tokens and pre-arrange every
    tensor into the exact SBUF tile layout the device kernel streams."""
    if DTYPE_MODE == "bf16":
        import ml_dtypes
        in_np = ml_dtypes.bfloat16
    else:
        in_np = np.float32
    in_maps = []
    for e in range(E):
        n = len(idx_e[e])
        idx = np.zeros(C, dtype=np.int64)
        idx[:n] = idx_e[e]
        s = np.zeros(C, dtype=np.float32)
        s[:n] = w_e[e]

        xe = x[idx]                                       # [C, D]
        x_t = np.ascontiguousarray(
            xe.reshape(C, DP, P).transpose(2, 1, 0), dtype=in_np
        )
        wg_t = np.ascontiguousarray(
            w_gate[e].reshape(IP, P, DP, P).transpose(0, 3, 2, 1), dtype=in_np
        )
        wu_t = np.ascontiguousarray(
            w_up[e].reshape(IP, P, DP, P).transpose(0, 3, 2, 1), dtype=in_np
        )
        wd_t = np.ascontiguousarray(
            w_down[e].reshape(DP, P, IP, P).transpose(0, 3, 2, 1), dtype=in_np
        )
        scl = np.ascontiguousarray(np.broadcast_to(s, (P, C)))
        in_maps.append(
            {"x_t": x_t, "wg_t": wg_t, "wu_t": wu_t, "wd_t": wd_t, "scl": scl}
        )
    return in_maps


def kernel(x, gate_w, w_gate, w_up, w_down):
    x = np.ascontiguousarray(np.asarray(x, dtype=np.float32))
    gate_w = np.asarray(gate_w, dtype=np.float32)
    w_gate = np.asarray(w_gate, dtype=np.float32)
    w_up = np.asarray(w_up, dtype=np.float32)
    w_down = np.asarray(w_down, dtype=np.float32)

    idx_e, w_e = _route(x, gate_w)
    C = _capacity(max(len(i) for i in idx_e))

    in_maps = prepare_in_maps(x, w_gate, w_up, w_down, idx_e, w_e, C)
    # Retry on transient device wedges (NRT_EXEC_UNIT_UNRECOVERABLE has been
    # observed sporadically on this fabric; a fresh dispatch recovers).
    last = None
    for _attempt in range(3):
        try:
            nc = build_kernel(C)
            res = run_bass_kernel_spmd(
                nc, in_maps, core_ids=list(range(N_CORES))
            )
            break
        except Exception as exc:  # noqa: BLE001
            last = exc
    else:
        raise last

    out = np.zeros((T, D), dtype=np.float32)
    for e in range(E):
        n = len(idx_e[e])
        if n == 0:
            continue
        y_t = res.results[e]["y_t"]                       # [P, DP, C]
        ye = y_t.transpose(2, 1, 0).reshape(C, D)[:n]     # [n, D]
        out[idx_e[e]] += ye
    return out


if __name__ == "__main__":
    rng = np.random.default_rng(0)
    # tiny smoke of the host routing path only
    print(_route(rng.standard_normal((16, D), dtype=np.float32),
                 rng.standard_normal((E, D), dtype=np.float32) * 0.02)[0])
